# revision 28
# baseline (speedup 1.0000x reference)
"""Trainium2 Bass kernel for Mobile2Former cross-attention block.

Computation (per batch b):
    xf   = x[b].reshape(C, H*W)                      # [64, 3136] keys=values
    q    = (z[b] @ Wq + bq).reshape(heads, M, C)     # [8, 6, 64]
    attn = softmax(q @ xf * C**-0.5, axis=-1)        # [8, 6, 3136]
    res  = attn @ xf.T                               # [8, 6, 64]
    out  = res.transpose(1,0,2).reshape(M, -1) @ Wo + bo + z[b]

Strategy: data-parallel over B across 8 cores (16 batches/core).  Batches are
processed in QUADS (4 batches) using fp8e4 DoubleRow matmuls: the contraction
dim packs two 128-row slabs (two batch-PAIRS block-diagonal for QK; two
consecutive 128-key chunks for AV), giving 2x tensor-engine throughput.  The
AV operand x^T arrives pre-transposed from the host (no on-chip transposes),
with a ones-column per slab yielding the softmax denominator for free.
Softmax runs without max subtraction (logits are O(1)).

The Scalar engine runs ONLY the exp waves (its ~20us of exp work is the
critical resource): per quad, 5 waves of 5 key-chunks land contiguously in
PSUM ([128, 960] f32; the one bank-crossing chunk is split into two matmuls)
so each exp is a single flat read.  The per-quad PE program runs QK waves
0-2, then the whole deferred AV/normalize of the previous quad, then waves
3-4, so the PE always has runnable work ahead of each exp-paced wait and exp
never stalls.  The q projection folds bias via a ones-row in the stationary;
one bulk DVE f32->fp8 conversion plus 8 small SBUF copies (vector+gpsimd)
build the block-diagonal moving operand.  All DMA issue is on sync; the Wo
projection runs 3/4 early, with only pairs 6-7 on the critical tail.
"""

import sys
from contextlib import ExitStack

import numpy as np

sys.path.insert(0, "/opt/trn_rl_repo")

import concourse.bass as bass
import concourse.tile as tile
from concourse import bacc as bacc_mod
from concourse import mybir
from concourse.bass_utils import run_bass_kernel_spmd

import ml_dtypes

BF16 = ml_dtypes.bfloat16
FP8 = ml_dtypes.float8_e4m3

N_CORES = 8
B, C, H, W = 128, 64, 56, 56
HW = H * W  # 3136
M, D = 6, 192
NH = 8
INNER = NH * C  # 512
BPC = B // N_CORES  # 16 batches per core
NQUAD = BPC // 4  # 4
NPAIR = BPC // 2  # 8
NCHUNK = (HW + 127) // 128  # 25 (24 full + one 64-wide)
NDC = (NCHUNK + 1) // 2  # 13 double-chunks (last has a dead slab)
PVW = 132  # AV moving cols per slab: 128 c + 1 ones + 3 pad

F32 = mybir.dt.float32
BF = mybir.dt.bfloat16
F8 = mybir.dt.float8e4
DR = mybir.MatmulPerfMode.DoubleRow
EXP = mybir.ActivationFunctionType.Exp

_CACHE = {}


def _build_nc() -> bass.Bass:
    nc = bacc_mod.Bacc()

    # QK stationary: [quad*128 part (2b x 64c), 25 j, 2 slab(pair), 128 n] fp8
    px_h = nc.declare_dram_parameter("px", [NQUAD * 128, NCHUNK * 2 * 128], F8,
                                     isOutput=False)
    # AV moving: [pair*128 part (n%128), 13 dc, 2 slab(n chunk), 132] fp8
    pv_h = nc.declare_dram_parameter("pv", [NPAIR * 128, NDC * 2 * PVW], F8,
                                     isOutput=False)
    # pk1 cols: [zt0 96][zt1+ones 96][wq0 512][wq1+bq 512] (rows 0:65 for 1-blks)
    pk1_h = nc.declare_dram_parameter("pk1", [128, 1216], F8, isOutput=False)
    # pk2 cols: [ident_bf 128][wo 4*192]
    pk2_h = nc.declare_dram_parameter("pk2", [128, 896], BF, isOutput=False)
    # z + bo in output layout: row 12r + 6bb + m
    zz_h = nc.declare_dram_parameter("zz", [96, D], F32, isOutput=False)
    out_h = nc.declare_dram_parameter("out", [96, D], F32, isOutput=True)

    with tile.TileContext(nc) as tc, ExitStack() as ctx:
        const = ctx.enter_context(tc.tile_pool(name="const", bufs=1))
        px_pool = ctx.enter_context(tc.tile_pool(name="px", bufs=4))
        pv_pool = ctx.enter_context(tc.tile_pool(name="pv", bufs=8))
        small = ctx.enter_context(tc.tile_pool(name="small", bufs=3))
        at_ps = ctx.enter_context(tc.tile_pool(name="at_ps", bufs=3, space="PSUM"))
        rs_ps = ctx.enter_context(tc.tile_pool(name="rs_ps", bufs=2, space="PSUM"))

        # ---------------- phase 0: warmup + constants ----------------
        # Tiny exp to trigger the ACT table load early (scalar idle til then).
        warm = const.tile([128, 8], F32)
        nc.vector.memset(warm, 0.0)
        warm2 = const.tile([128, 8], F32)
        nc.scalar.activation(out=warm2, in_=warm, func=EXP)

        # All loads ride one sync-queue FIFO in need-order: the DMA engines
        # round-robin across queued transfers, so queue order IS priority.
        pk1 = const.tile([128, 1216], F8)
        nc.sync.dma_start(out=pk1, in_=pk1_h.ap())
        zt0 = pk1[:, 0:96]
        zt1 = pk1[0:65, 96:192]
        wq0 = pk1[:, 192:704]
        wq1 = pk1[0:65, 704:1216]

        px_t = []
        for g in range(NQUAD):
            px_t.append(px_pool.tile([128, NCHUNK * 2 * 128], F8, tag="px",
                                     name=f"px{g}"))
        pv_t = []
        for r in range(NPAIR):
            pv_t.append(pv_pool.tile([128, NDC * 2 * PVW], F8, tag="pv",
                                     name=f"pv{r}"))

        def load_px(g, eng=None):
            (eng or nc.sync).dma_start(
                out=px_t[g], in_=px_h.ap()[128 * g: 128 * (g + 1), :])

        def load_pv(r, eng=None):
            (eng or nc.sync).dma_start(
                out=pv_t[r], in_=pv_h.ap()[128 * r: 128 * (r + 1), :])

        # one global priority order; full buffering means no issue ever
        # blocks the queue on a tile-reuse wait
        load_px(0)
        load_px(1)
        load_pv(0)
        load_pv(1)
        load_px(2)
        pk2 = const.tile([128, 896], BF)
        nc.sync.dma_start(out=pk2, in_=pk2_h.ap())
        ident_bf = pk2[:, 0:128]
        wo_sb = pk2[:, 128:896]
        load_pv(2)
        load_pv(3)
        load_px(3)
        load_pv(4)
        load_pv(5)
        zz_sb = const.tile([96, D], F32)
        nc.sync.dma_start(out=zz_sb, in_=zz_h.ap())
        load_pv(6)
        load_pv(7)

        # ---------------- qproj ----------------
        # qp[64gh+c, 96ii + 6bl + t] = q^T[128ii + 64gh + c, batch bl, row t]
        # (bias folded: zt1 row 64 = ones, wq1 row 64 = bq*scale)
        qp = rs_ps.tile([128, 4 * 96], F32, tag="rs", name="qp")
        qp_g = qp.rearrange("p (ii x) -> p ii x", ii=4)
        for ii in range(4):
            nc.tensor.matmul(
                qp_g[:, ii, :], lhsT=wq0[:, 128 * ii: 128 * ii + 128], rhs=zt0,
                start=True, stop=False,
            )
            nc.tensor.matmul(
                qp_g[:, ii, :], lhsT=wq1[:, 128 * ii: 128 * ii + 128], rhs=zt1,
                start=False, stop=True,
            )
        # bulk f32 -> fp8 conversion (split by partition half so the gh=0
        # shuffles can start early), then small SBUF shuffles
        qa = const.tile([128, 384], F8)
        nc.vector.tensor_copy(out=qa[0:64, :], in_=qp[0:64, :])
        nc.scalar.activation(out=qa[64:128, :], in_=qp[64:128, :],
                             func=mybir.ActivationFunctionType.Copy)
        qa_r = qa.rearrange("p (ii g i b2 t) -> p g t ii i b2",
                            ii=4, g=NQUAD, i=2, b2=2)

        # qT4big: QK moving operand, [128 (c2), 4 g, 2 slab(pair), 192] fp8;
        # valid block of quad g, slab i: cols 96i:96i+96 (block-diagonal).
        # In-block col = 48bb + 8t + 2ii + gh; flat offset within a g-block is
        # 288i + 48bb + gh + 8t + 2ii.
        # one tile per quad: QK(g) then waits only on quad g's own shuffles
        qT4_t = []
        for g in range(NQUAD):
            t = const.tile([128, 2 * 192], F8, name=f"qT4_{g}")
            nc.gpsimd.memset(t, 0.0)
            qT4_t.append(t)
        for g in range(NQUAD):  # quad 0's copies first so QK starts early
            for i in range(2):
                for bb in range(2):
                    for gh in range(2):
                        base = 288 * i + 48 * bb
                        dst = qT4_t[g][64 * bb: 64 * bb + 64, base: base + 48]
                        dst = dst.rearrange("p (t ii w) -> p t ii w", t=6, ii=4)
                        dst = dst[:, :, :, gh]
                        src = qa_r[64 * gh: 64 * gh + 64, g, :, :, i, bb]
                        eng = nc.gpsimd if gh == 0 else nc.vector
                        eng.tensor_copy(out=dst, in_=src)

        # ax buffers: exp output / AV stationary, [128, 13 dc, 2 slab, 192] fp8.
        # Dead tail region (dc12 slab1) pre-zeroed once; exp never writes it.
        ax_bufs = []
        for i in range(2):
            t = const.tile([128, NDC * 2 * 192], F8, name=f"ax_buf{i}")
            tv = t.rearrange("p (d i c) -> p d i c", d=NDC, i=2)
            nc.gpsimd.memset(tv[64:128, NDC - 1, 0, :], 0.0)
            nc.gpsimd.memset(tv[:, NDC - 1, 1, :], 0.0)
            ax_bufs.append(t)

        # fcl_all: Wo-projection stationary for all 8 pairs,
        # fcl_all[64*hl + c, kk, 12*r + 6*bb + m] bf16
        fcl_all = const.tile([128, 4 * 96], BF)
        fcl_g = fcl_all.rearrange("q (kk x) -> q kk x", kk=4)
        out_sb = const.tile([96, D], F32)

        # ---------------- per-quad pieces ----------------
        def do_qk_waves(g, ats, waves):
            pxv = px_t[g].rearrange("p (j i t) -> p j i t", j=NCHUNK, i=2)
            qT4v = qT4_t[g].rearrange("p (i c) -> p i c", i=2)
            for w in waves:
                at = at_ps.tile([128, 960], F32, tag="at", name=f"at{g}_{w}")
                ats[w] = at
                for jj in range(5):
                    j = 5 * w + jj
                    cw = 64 if j == NCHUNK - 1 else 128
                    if jj == 2:  # split at the PSUM bank boundary (el 512)
                        nc.tensor.matmul(
                            at[0:cw, 384:512], lhsT=pxv[:, j, :, 0:cw],
                            rhs=qT4v[:, :, 0:128], perf_mode=DR,
                            start=True, stop=True,
                        )
                        nc.tensor.matmul(
                            at[0:cw, 512:576], lhsT=pxv[:, j, :, 0:cw],
                            rhs=qT4v[:, :, 128:192], perf_mode=DR,
                            start=True, stop=True,
                        )
                    else:
                        o = 192 * jj
                        nc.tensor.matmul(
                            at[0:cw, o: o + 192], lhsT=pxv[:, j, :, 0:cw],
                            rhs=qT4v, perf_mode=DR, start=True, stop=True,
                        )

        def do_exp(g, ats, axf):
            for w in range(5):
                nc.scalar.activation(
                    out=axf[:, 960 * w: 960 * (w + 1)], in_=ats[w], func=EXP,
                )

        def do_av_all(p, d0=0, d1=NDC):
            g, axv, rsum = p["g"], p["axv"], p["rsum"]
            for d in range(d0, d1):
                for i in range(2):
                    nc.tensor.matmul(
                        rsum[i], lhsT=axv[:, d, :, 96 * i: 96 * i + 96],
                        rhs=pv_t[2 * g + i].rearrange(
                            "p (d i c) -> p d i c", d=NDC, i=2)[:, d, :, :],
                        perf_mode=DR, start=(d == 0), stop=(d == NDC - 1),
                    )

        def do_norm(p):
            # normalize both pairs, transpose into one tile, 4 merged fcl
            # copies: fcl[64hl+c, kk, 12r+6bb+m] = rtb[64bb+c, 96i+48bb+12kk+6hl+m]
            g, rsum = p["g"], p["rsum"]
            rtb = p["rsb"][:, 264:360].bitcast(BF)
            for i in range(2):
                r = 2 * g + i
                inv = small.tile([96, 1], F32, tag="inv", name=f"inv{r}")
                nc.vector.reciprocal(out=inv, in_=rsum[i][:, 128:129])
                r2n = small.tile([96, 128], BF, tag="r2n", name=f"r2n{r}")
                nc.vector.tensor_scalar_mul(out=r2n, in0=rsum[i][:, 0:128],
                                            scalar1=inv)
                nc.tensor.transpose(rtb[:, 96 * i: 96 * i + 96], r2n,
                                    ident_bf[0:96, 0:96])
            rt_v = rtb.rearrange("q (i b2 kk h2 m) -> q kk i b2 h2 m",
                                 i=2, b2=2, kk=4, h2=2)
            for hl in range(2):
                for bb in range(2):
                    dst = fcl_g[64 * hl: 64 * hl + 64, :, 24 * g: 24 * g + 24]
                    dst = dst.rearrange("p kk (i b2 m) -> p kk i b2 m",
                                        i=2, b2=2)[:, :, :, bb, :]
                    src = rt_v[64 * bb: 64 * bb + 64, :, :, bb, hl, :]
                    nc.vector.tensor_copy(out=dst, in_=src)

        def do_wo(r0, nr, o2):
            # out rows r0 .. r0+nr of the Wo projection + residual
            sl = slice(r0, r0 + nr)
            for kk in range(4):
                nc.tensor.matmul(
                    out=o2, lhsT=fcl_g[:, kk, sl],
                    rhs=wo_sb[:, 192 * kk: 192 * kk + 192],
                    start=(kk == 0), stop=(kk == 3),
                )
            nc.vector.tensor_add(out=out_sb[sl, :], in0=o2, in1=zz_sb[sl, :])
            nc.sync.dma_start(out=out_h.ap()[sl, :], in_=out_sb[sl, :])

        # ---------------- main loop ----------------
        pend = {}
        for g in range(NQUAD):
            ats = {}
            do_qk_waves(g, ats, [0, 1])
            if pend:
                do_av_all(pend)
            do_qk_waves(g, ats, [2, 3, 4])
            if pend:
                do_norm(pend)

            ax = ax_bufs[g % 2]
            axv = ax.rearrange("p (d i c) -> p d i c", d=NDC, i=2)
            do_exp(g, ats, ax)

            rsb = rs_ps.tile([128, 360], F32, tag="rs", name=f"rsum{g}")
            pend = {"g": g, "axv": axv, "rsb": rsb,
                    "rsum": [rsb[0:96, 0:PVW], rsb[0:96, PVW: 2 * PVW]]}

        do_av_all(pend, 0, 10)
        o2a = rs_ps.tile([64, D], F32, tag="rs", name="o2a")
        do_wo(0, 64, o2a)
        do_av_all(pend, 10, NDC)
        do_norm(pend)
        o2b = rs_ps.tile([32, D], F32, tag="rs", name="o2b")
        do_wo(64, 32, o2b)

    return nc


def get_nc() -> bass.Bass:
    if "nc" not in _CACHE:
        nc = _build_nc()
        # The PJRT exec path serializes nc.m as-is; run Bacc's legalization
        # (wait splitting, register allocation, ...) explicitly.
        nc.finalize()
        _CACHE["nc"] = nc
    return _CACHE["nc"]


def make_in_maps(x, z, Wq, bq, Wo, bo):
    """Host-side prep + sharding into per-core input maps."""
    x = np.asarray(x, dtype=np.float32)
    z = np.asarray(z, dtype=np.float32)
    Wq = np.asarray(Wq, dtype=np.float32)
    bq = np.asarray(bq, dtype=np.float32)
    Wo = np.asarray(Wo, dtype=np.float32)
    bo = np.asarray(bo, dtype=np.float32)

    scale = np.float32(C ** -0.5)
    x_f8 = x.reshape(B, C, HW).astype(FP8)
    wq_s = (Wq * scale).astype(BF16)
    bq_s = (bq * scale).astype(BF16)
    wo_bf = Wo.astype(BF16)
    # pk2 = [ident 128 | wo 4*192] with wo[p, 192k+d] = Wo[128k+p, d]
    pk2 = np.zeros((128, 896), dtype=BF16)
    pk2[:, 0:128] = np.eye(128, dtype=BF16)
    pk2[:, 128:896] = np.ascontiguousarray(
        wo_bf.reshape(4, 128, D).transpose(1, 0, 2).reshape(128, 4 * D)
    )

    in_maps = []
    for ci in range(N_CORES):
        s = slice(ci * BPC, (ci + 1) * BPC)
        xc = x_f8[s]  # [16, 64, 3136]

        # px: QK stationary. px[g, 64bb+c, j, i, t] = x[4g+2i+bb, c, 128j+t]
        xp = np.zeros((BPC, C, NCHUNK, 128), dtype=FP8)
        xp[:, :, :24, :] = xc[:, :, : 24 * 128].reshape(BPC, C, 24, 128)
        xp[:, :, 24, :64] = xc[:, :, 24 * 128:]
        xq = xp.reshape(NQUAD, 2, 2, C, NCHUNK, 128)  # [g, i, bb, c, j, t]
        px = np.ascontiguousarray(xq.transpose(0, 2, 3, 4, 1, 5)).reshape(
            NQUAD * 128, NCHUNK * 2 * 128
        )

        # pv: AV moving (x^T with ones col).
        # pv[r, t, d, i, cc] = x[2r + cc//64, cc%64, 256d + 128i + t]
        xt_pad = np.zeros((NPAIR, NDC * 256, PVW), dtype=FP8)
        xt_pad[:, :HW, :128] = (
            xc.reshape(NPAIR, 2, C, HW).transpose(0, 3, 1, 2).reshape(NPAIR, HW, 128)
        )
        xt_pad[:, :HW, 128] = np.float32(1.0)
        pv = np.ascontiguousarray(
            xt_pad.reshape(NPAIR, NDC, 2, 128, PVW).transpose(0, 3, 1, 2, 4)
        ).reshape(NPAIR * 128, NDC * 2 * PVW)

        # zt[d, 6*b_local + m] = z[core_base + b_local, m, d]; bias folded via
        # ones row (zt1 row 64 = 1, wq1 row 64 = bq*scale)
        zt = z[s].reshape(BPC * M, D).T.astype(FP8)
        pk1 = np.zeros((128, 1216), dtype=FP8)
        pk1[:, 0:96] = zt[0:128]
        pk1[0:64, 96:192] = zt[128:192]
        pk1[64, 96:192] = np.float32(1.0)
        pk1[:, 192:704] = wq_s[0:128].astype(FP8)
        pk1[0:64, 704:1216] = wq_s[128:192].astype(FP8)
        pk1[64, 704:1216] = bq_s.astype(FP8)

        # zz[12r + 6bb + m] = z[2r + bb, m] + bo
        zz = (z[s] + bo[None, None, :]).reshape(96, D).astype(np.float32)

        in_maps.append({"px": px, "pv": pv, "pk1": pk1, "pk2": pk2, "zz": zz})
    return in_maps


def kernel(**inputs) -> np.ndarray:
    nc = get_nc()
    in_maps = make_in_maps(
        inputs["x"], inputs["z"], inputs["Wq"], inputs["bq"],
        inputs["Wo"], inputs["bo"],
    )
    res = run_bass_kernel_spmd(nc, in_maps, list(range(N_CORES)))
    out = np.concatenate(
        [
            np.asarray(res.results[i]["out"]).reshape(BPC, M, D)
            for i in range(N_CORES)
        ],
        axis=0,
    )
    return out.astype(np.float32)


# revision 29
# speedup vs baseline: 1.0038x; 1.0038x over previous
"""Trainium2 Bass kernel for Mobile2Former cross-attention block.

Computation (per batch b):
    xf   = x[b].reshape(C, H*W)                      # [64, 3136] keys=values
    q    = (z[b] @ Wq + bq).reshape(heads, M, C)     # [8, 6, 64]
    attn = softmax(q @ xf * C**-0.5, axis=-1)        # [8, 6, 3136]
    res  = attn @ xf.T                               # [8, 6, 64]
    out  = res.transpose(1,0,2).reshape(M, -1) @ Wo + bo + z[b]

Strategy: data-parallel over B across 8 cores (16 batches/core).  Batches are
processed in QUADS (4 batches) using fp8e4 DoubleRow matmuls: the contraction
dim packs two 128-row slabs (two batch-PAIRS block-diagonal for QK; two
consecutive 128-key chunks for AV), giving 2x tensor-engine throughput.  The
AV operand x^T arrives pre-transposed from the host (no on-chip transposes),
with a ones-column per slab yielding the softmax denominator for free.
Softmax runs without max subtraction (logits are O(1)).

The Scalar engine runs ONLY the exp waves (its ~20us of exp work is the
critical resource): per quad, 5 waves of 5 key-chunks land contiguously in
PSUM ([128, 960] f32; the one bank-crossing chunk is split into two matmuls)
so each exp is a single flat read.  The per-quad PE program runs QK waves
0-2, then the whole deferred AV/normalize of the previous quad, then waves
3-4, so the PE always has runnable work ahead of each exp-paced wait and exp
never stalls.  The q projection folds bias via a ones-row in the stationary;
one bulk DVE f32->fp8 conversion plus 8 small SBUF copies (vector+gpsimd)
build the block-diagonal moving operand.  All DMA issue is on sync; the Wo
projection runs 3/4 early, with only pairs 6-7 on the critical tail.
"""

import sys
from contextlib import ExitStack

import numpy as np

sys.path.insert(0, "/opt/trn_rl_repo")

import concourse.bass as bass
import concourse.tile as tile
from concourse import bacc as bacc_mod
from concourse import mybir
from concourse.bass_utils import run_bass_kernel_spmd

import ml_dtypes

BF16 = ml_dtypes.bfloat16
FP8 = ml_dtypes.float8_e4m3

N_CORES = 8
B, C, H, W = 128, 64, 56, 56
HW = H * W  # 3136
M, D = 6, 192
NH = 8
INNER = NH * C  # 512
BPC = B // N_CORES  # 16 batches per core
NQUAD = BPC // 4  # 4
NPAIR = BPC // 2  # 8
NCHUNK = (HW + 127) // 128  # 25 (24 full + one 64-wide)
NDC = (NCHUNK + 1) // 2  # 13 double-chunks (last has a dead slab)
PVW = 132  # AV moving cols per slab: 128 c + 1 ones + 3 pad

F32 = mybir.dt.float32
BF = mybir.dt.bfloat16
F8 = mybir.dt.float8e4
DR = mybir.MatmulPerfMode.DoubleRow
EXP = mybir.ActivationFunctionType.Exp

_CACHE = {}


def _build_nc() -> bass.Bass:
    nc = bacc_mod.Bacc()

    # QK stationary: [quad*128 part (2b x 64c), 25 j, 2 slab(pair), 128 n] fp8
    px_h = nc.declare_dram_parameter("px", [NQUAD * 128, NCHUNK * 2 * 128], F8,
                                     isOutput=False)
    # AV moving: [pair*128 part (n%128), 13 dc, 2 slab(n chunk), 132] fp8
    pv_h = nc.declare_dram_parameter("pv", [NPAIR * 128, NDC * 2 * PVW], F8,
                                     isOutput=False)
    # pk1 cols: [zt0 96][zt1+ones 96][wq0 512][wq1+bq 512] (rows 0:65 for 1-blks)
    pk1_h = nc.declare_dram_parameter("pk1", [128, 1216], F8, isOutput=False)
    # pk2 cols: [ident_bf 128][wo 4*192]
    pk2_h = nc.declare_dram_parameter("pk2", [128, 896], BF, isOutput=False)
    # z + bo in output layout: row 12r + 6bb + m
    zz_h = nc.declare_dram_parameter("zz", [96, D], F32, isOutput=False)
    out_h = nc.declare_dram_parameter("out", [96, D], F32, isOutput=True)

    with tile.TileContext(nc) as tc, ExitStack() as ctx:
        const = ctx.enter_context(tc.tile_pool(name="const", bufs=1))
        px_pool = ctx.enter_context(tc.tile_pool(name="px", bufs=4))
        pv_pool = ctx.enter_context(tc.tile_pool(name="pv", bufs=8))
        small = ctx.enter_context(tc.tile_pool(name="small", bufs=3))
        at_ps = ctx.enter_context(tc.tile_pool(name="at_ps", bufs=3, space="PSUM"))
        rs_ps = ctx.enter_context(tc.tile_pool(name="rs_ps", bufs=2, space="PSUM"))

        # ---------------- phase 0: warmup + constants ----------------
        # Tiny exp to trigger the ACT table load early (scalar idle til then).
        warm = const.tile([128, 8], F32)
        nc.vector.memset(warm, 0.0)
        warm2 = const.tile([128, 8], F32)
        nc.scalar.activation(out=warm2, in_=warm, func=EXP)

        # All loads ride one sync-queue FIFO in need-order: the DMA engines
        # round-robin across queued transfers, so queue order IS priority.
        pk1 = const.tile([128, 1216], F8)
        nc.sync.dma_start(out=pk1, in_=pk1_h.ap())
        zt0 = pk1[:, 0:96]
        zt1 = pk1[0:65, 96:192]
        wq0 = pk1[:, 192:704]
        wq1 = pk1[0:65, 704:1216]

        px_t = []
        for g in range(NQUAD):
            px_t.append(px_pool.tile([128, NCHUNK * 2 * 128], F8, tag="px",
                                     name=f"px{g}"))
        pv_t = []
        for r in range(NPAIR):
            pv_t.append(pv_pool.tile([128, NDC * 2 * PVW], F8, tag="pv",
                                     name=f"pv{r}"))

        def load_px(g, eng=None):
            (eng or nc.sync).dma_start(
                out=px_t[g], in_=px_h.ap()[128 * g: 128 * (g + 1), :])

        def load_pv(r, eng=None):
            (eng or nc.sync).dma_start(
                out=pv_t[r], in_=pv_h.ap()[128 * r: 128 * (r + 1), :])

        # one global priority order; full buffering means no issue ever
        # blocks the queue on a tile-reuse wait
        load_px(0)
        load_px(1)
        load_pv(0)
        load_pv(1)
        load_px(2)
        pk2 = const.tile([128, 896], BF)
        nc.sync.dma_start(out=pk2, in_=pk2_h.ap())
        ident_bf = pk2[:, 0:128]
        wo_sb = pk2[:, 128:896]
        load_pv(2)
        load_pv(3)
        load_px(3)
        load_pv(4)
        load_pv(5)
        zz_sb = const.tile([96, D], F32)
        nc.sync.dma_start(out=zz_sb, in_=zz_h.ap())
        load_pv(6)
        load_pv(7)

        # ---------------- qproj ----------------
        # qp[64gh+c, 96ii + 6bl + t] = q^T[128ii + 64gh + c, batch bl, row t]
        # (bias folded: zt1 row 64 = ones, wq1 row 64 = bq*scale)
        qp = rs_ps.tile([128, 4 * 96], F32, tag="rs", name="qp")
        qp_g = qp.rearrange("p (ii x) -> p ii x", ii=4)
        for ii in range(4):
            nc.tensor.matmul(
                qp_g[:, ii, :], lhsT=wq0[:, 128 * ii: 128 * ii + 128], rhs=zt0,
                start=True, stop=False,
            )
            nc.tensor.matmul(
                qp_g[:, ii, :], lhsT=wq1[:, 128 * ii: 128 * ii + 128], rhs=zt1,
                start=False, stop=True,
            )
        # bulk f32 -> fp8 conversion (split by partition half so the gh=0
        # shuffles can start early), then small SBUF shuffles
        qa = const.tile([128, 384], F8)
        nc.vector.tensor_copy(out=qa[0:64, :], in_=qp[0:64, :])
        nc.scalar.activation(out=qa[64:128, :], in_=qp[64:128, :],
                             func=mybir.ActivationFunctionType.Copy)
        qa_r = qa.rearrange("p (ii g i b2 t) -> p g t ii i b2",
                            ii=4, g=NQUAD, i=2, b2=2)

        # qT4big: QK moving operand, [128 (c2), 4 g, 2 slab(pair), 192] fp8;
        # valid block of quad g, slab i: cols 96i:96i+96 (block-diagonal).
        # In-block col = 48bb + 8t + 2ii + gh; flat offset within a g-block is
        # 288i + 48bb + gh + 8t + 2ii.
        # one tile per quad: QK(g) then waits only on quad g's own shuffles
        qT4_t = []
        for g in range(NQUAD):
            t = const.tile([128, 2 * 192], F8, name=f"qT4_{g}")
            nc.gpsimd.memset(t, 0.0)
            qT4_t.append(t)
        for g in range(NQUAD):  # quad 0's copies first so QK starts early
            for i in range(2):
                for bb in range(2):
                    for gh in range(2):
                        base = 288 * i + 48 * bb
                        dst = qT4_t[g][64 * bb: 64 * bb + 64, base: base + 48]
                        dst = dst.rearrange("p (t ii w) -> p t ii w", t=6, ii=4)
                        dst = dst[:, :, :, gh]
                        src = qa_r[64 * gh: 64 * gh + 64, g, :, :, i, bb]
                        eng = nc.gpsimd if gh == 0 else nc.vector
                        eng.tensor_copy(out=dst, in_=src)

        # ax buffers: exp output / AV stationary, [128, 13 dc, 2 slab, 192] fp8.
        # Dead tail region (dc12 slab1) pre-zeroed once; exp never writes it.
        ax_bufs = []
        for i in range(2):
            t = const.tile([128, NDC * 2 * 192], F8, name=f"ax_buf{i}")
            tv = t.rearrange("p (d i c) -> p d i c", d=NDC, i=2)
            nc.gpsimd.memset(tv[64:128, NDC - 1, 0, :], 0.0)
            nc.gpsimd.memset(tv[:, NDC - 1, 1, :], 0.0)
            ax_bufs.append(t)

        # fcl_all: Wo-projection stationary for all 8 pairs,
        # fcl_all[64*hl + c, kk, 12*r + 6*bb + m] bf16
        fcl_all = const.tile([128, 4 * 96], BF)
        fcl_g = fcl_all.rearrange("q (kk x) -> q kk x", kk=4)
        out_sb = const.tile([96, D], F32)

        # ---------------- per-quad pieces ----------------
        def do_qk_waves(g, ats, waves):
            pxv = px_t[g].rearrange("p (j i t) -> p j i t", j=NCHUNK, i=2)
            qT4v = qT4_t[g].rearrange("p (i c) -> p i c", i=2)
            for w in waves:
                at = at_ps.tile([128, 960], F32, tag="at", name=f"at{g}_{w}")
                ats[w] = at
                for jj in range(5):
                    j = 5 * w + jj
                    cw = 64 if j == NCHUNK - 1 else 128
                    if jj == 2:  # split at the PSUM bank boundary (el 512)
                        nc.tensor.matmul(
                            at[0:cw, 384:512], lhsT=pxv[:, j, :, 0:cw],
                            rhs=qT4v[:, :, 0:128], perf_mode=DR,
                            start=True, stop=True,
                        )
                        nc.tensor.matmul(
                            at[0:cw, 512:576], lhsT=pxv[:, j, :, 0:cw],
                            rhs=qT4v[:, :, 128:192], perf_mode=DR,
                            start=True, stop=True,
                        )
                    else:
                        o = 192 * jj
                        nc.tensor.matmul(
                            at[0:cw, o: o + 192], lhsT=pxv[:, j, :, 0:cw],
                            rhs=qT4v, perf_mode=DR, start=True, stop=True,
                        )

        def do_exp(g, ats, axf):
            for w in range(5):
                nc.scalar.activation(
                    out=axf[:, 960 * w: 960 * (w + 1)], in_=ats[w], func=EXP,
                )

        def do_av_all(p, d0=0, d1=NDC):
            g, axv, rsum = p["g"], p["axv"], p["rsum"]
            for d in range(d0, d1):
                for i in range(2):
                    nc.tensor.matmul(
                        rsum[i], lhsT=axv[:, d, :, 96 * i: 96 * i + 96],
                        rhs=pv_t[2 * g + i].rearrange(
                            "p (d i c) -> p d i c", d=NDC, i=2)[:, d, :, :],
                        perf_mode=DR, start=(d == 0), stop=(d == NDC - 1),
                    )

        def do_norm(p):
            # normalize both pairs, transpose into one tile, 4 merged fcl
            # copies: fcl[64hl+c, kk, 12r+6bb+m] = rtb[64bb+c, 96i+48bb+12kk+6hl+m]
            g, rsum = p["g"], p["rsum"]
            rtb = p["rsb"][:, 264:360].bitcast(BF)
            for i in range(2):
                r = 2 * g + i
                inv = small.tile([96, 1], F32, tag="inv", name=f"inv{r}")
                nc.vector.reciprocal(out=inv, in_=rsum[i][:, 128:129])
                r2n = small.tile([96, 128], BF, tag="r2n", name=f"r2n{r}")
                nc.vector.tensor_scalar_mul(out=r2n, in0=rsum[i][:, 0:128],
                                            scalar1=inv)
                nc.tensor.transpose(rtb[:, 96 * i: 96 * i + 96], r2n,
                                    ident_bf[0:96, 0:96])
            rt_v = rtb.rearrange("q (i b2 kk h2 m) -> q kk i b2 h2 m",
                                 i=2, b2=2, kk=4, h2=2)
            for hl in range(2):
                for bb in range(2):
                    dst = fcl_g[64 * hl: 64 * hl + 64, :, 24 * g: 24 * g + 24]
                    dst = dst.rearrange("p kk (i b2 m) -> p kk i b2 m",
                                        i=2, b2=2)[:, :, :, bb, :]
                    src = rt_v[64 * bb: 64 * bb + 64, :, :, bb, hl, :]
                    nc.vector.tensor_copy(out=dst, in_=src)

        def do_wo(r0, nr, o2):
            # out rows r0 .. r0+nr of the Wo projection + residual
            sl = slice(r0, r0 + nr)
            for kk in range(4):
                nc.tensor.matmul(
                    out=o2, lhsT=fcl_g[:, kk, sl],
                    rhs=wo_sb[:, 192 * kk: 192 * kk + 192],
                    start=(kk == 0), stop=(kk == 3),
                )
            nc.vector.tensor_add(out=out_sb[sl, :], in0=o2, in1=zz_sb[sl, :])
            nc.sync.dma_start(out=out_h.ap()[sl, :], in_=out_sb[sl, :])

        # ---------------- main loop ----------------
        pend = {}
        for g in range(NQUAD):
            ats = {}
            do_qk_waves(g, ats, [0, 1])
            if pend:
                do_av_all(pend)
            do_qk_waves(g, ats, [2, 3, 4])
            if pend:
                do_norm(pend)
                if pend["g"] == 2:
                    o2a = rs_ps.tile([64, D], F32, tag="rs", name="o2a")
                    do_wo(0, 64, o2a)

            ax = ax_bufs[g % 2]
            axv = ax.rearrange("p (d i c) -> p d i c", d=NDC, i=2)
            do_exp(g, ats, ax)

            rsb = rs_ps.tile([128, 360], F32, tag="rs", name=f"rsum{g}")
            pend = {"g": g, "axv": axv, "rsb": rsb,
                    "rsum": [rsb[0:96, 0:PVW], rsb[0:96, PVW: 2 * PVW]]}

        do_av_all(pend)
        do_norm(pend)
        o2b = rs_ps.tile([32, D], F32, tag="rs", name="o2b")
        do_wo(64, 32, o2b)

    return nc


def get_nc() -> bass.Bass:
    if "nc" not in _CACHE:
        nc = _build_nc()
        # The PJRT exec path serializes nc.m as-is; run Bacc's legalization
        # (wait splitting, register allocation, ...) explicitly.
        nc.finalize()
        _CACHE["nc"] = nc
    return _CACHE["nc"]


def make_in_maps(x, z, Wq, bq, Wo, bo):
    """Host-side prep + sharding into per-core input maps."""
    x = np.asarray(x, dtype=np.float32)
    z = np.asarray(z, dtype=np.float32)
    Wq = np.asarray(Wq, dtype=np.float32)
    bq = np.asarray(bq, dtype=np.float32)
    Wo = np.asarray(Wo, dtype=np.float32)
    bo = np.asarray(bo, dtype=np.float32)

    scale = np.float32(C ** -0.5)
    x_f8 = x.reshape(B, C, HW).astype(FP8)
    wq_s = (Wq * scale).astype(BF16)
    bq_s = (bq * scale).astype(BF16)
    wo_bf = Wo.astype(BF16)
    # pk2 = [ident 128 | wo 4*192] with wo[p, 192k+d] = Wo[128k+p, d]
    pk2 = np.zeros((128, 896), dtype=BF16)
    pk2[:, 0:128] = np.eye(128, dtype=BF16)
    pk2[:, 128:896] = np.ascontiguousarray(
        wo_bf.reshape(4, 128, D).transpose(1, 0, 2).reshape(128, 4 * D)
    )

    in_maps = []
    for ci in range(N_CORES):
        s = slice(ci * BPC, (ci + 1) * BPC)
        xc = x_f8[s]  # [16, 64, 3136]

        # px: QK stationary. px[g, 64bb+c, j, i, t] = x[4g+2i+bb, c, 128j+t]
        xp = np.zeros((BPC, C, NCHUNK, 128), dtype=FP8)
        xp[:, :, :24, :] = xc[:, :, : 24 * 128].reshape(BPC, C, 24, 128)
        xp[:, :, 24, :64] = xc[:, :, 24 * 128:]
        xq = xp.reshape(NQUAD, 2, 2, C, NCHUNK, 128)  # [g, i, bb, c, j, t]
        px = np.ascontiguousarray(xq.transpose(0, 2, 3, 4, 1, 5)).reshape(
            NQUAD * 128, NCHUNK * 2 * 128
        )

        # pv: AV moving (x^T with ones col).
        # pv[r, t, d, i, cc] = x[2r + cc//64, cc%64, 256d + 128i + t]
        xt_pad = np.zeros((NPAIR, NDC * 256, PVW), dtype=FP8)
        xt_pad[:, :HW, :128] = (
            xc.reshape(NPAIR, 2, C, HW).transpose(0, 3, 1, 2).reshape(NPAIR, HW, 128)
        )
        xt_pad[:, :HW, 128] = np.float32(1.0)
        pv = np.ascontiguousarray(
            xt_pad.reshape(NPAIR, NDC, 2, 128, PVW).transpose(0, 3, 1, 2, 4)
        ).reshape(NPAIR * 128, NDC * 2 * PVW)

        # zt[d, 6*b_local + m] = z[core_base + b_local, m, d]; bias folded via
        # ones row (zt1 row 64 = 1, wq1 row 64 = bq*scale)
        zt = z[s].reshape(BPC * M, D).T.astype(FP8)
        pk1 = np.zeros((128, 1216), dtype=FP8)
        pk1[:, 0:96] = zt[0:128]
        pk1[0:64, 96:192] = zt[128:192]
        pk1[64, 96:192] = np.float32(1.0)
        pk1[:, 192:704] = wq_s[0:128].astype(FP8)
        pk1[0:64, 704:1216] = wq_s[128:192].astype(FP8)
        pk1[64, 704:1216] = bq_s.astype(FP8)

        # zz[12r + 6bb + m] = z[2r + bb, m] + bo
        zz = (z[s] + bo[None, None, :]).reshape(96, D).astype(np.float32)

        in_maps.append({"px": px, "pv": pv, "pk1": pk1, "pk2": pk2, "zz": zz})
    return in_maps


def kernel(**inputs) -> np.ndarray:
    nc = get_nc()
    in_maps = make_in_maps(
        inputs["x"], inputs["z"], inputs["Wq"], inputs["bq"],
        inputs["Wo"], inputs["bo"],
    )
    res = run_bass_kernel_spmd(nc, in_maps, list(range(N_CORES)))
    out = np.concatenate(
        [
            np.asarray(res.results[i]["out"]).reshape(BPC, M, D)
            for i in range(N_CORES)
        ],
        axis=0,
    )
    return out.astype(np.float32)
